# revision 48
# baseline (speedup 1.0000x reference)
"""Trainium2 Bass kernel for nn_BERT_pool_mutil_avr (cosine-attention + ROI pool + conv).

Single fused launch. Sharding: core pair (2b, 2b+1) both take batch b with the
full T=2048 tokens; core 2b owns heads {0,1,6,7} (conv scales nb=1,9), core
2b+1 owns heads {2,3,4,5} (scales nb=3,7). The conv is block-diagonal over
scale groups, so each core pools and convolves its ROIs completely locally —
no cross-core exchange. 10 pooling bins per ROI on every core (balanced).
The conv enumerates all (head, bin) pairs with zero-padded weights so the
instruction stream is identical on every core (SPMD).

K is projected in [token, channel] layout (same as V): per-token dot(q,k) and
|k|^2 come from free-dim reductions (activation/stt accum_out), so attention
probs are born in column form — no PE transposes, no row-space math.
"""
import os
import numpy as np
import ml_dtypes

import concourse.bass as bass
import concourse.mybir as mybir
import concourse.tile as tile
from concourse import bacc, bass_utils
from concourse.masks import make_identity

F32 = mybir.dt.float32
F32R = mybir.dt.float32r
BF16 = mybir.dt.bfloat16
F16 = mybir.dt.float16
F8 = mybir.dt.float8e4
I32 = mybir.dt.int32
AF = mybir.ActivationFunctionType
OP = mybir.AluOpType

TRACE = bool(int(os.environ.get("KTRACE", "0")))
LAST_EXEC_NS = 0
LAST_RES = []

B, D, T, NROI, H, DK = 4, 1024, 2048, 128, 8, 128
KT = D // 128          # 8 contraction tiles
MT = T // 128          # 16 token tiles
HC = 4                 # local heads per core
CH = HC * DK           # 512 local channels
NBK = 10               # pooling bins per roi per core
HSETS = [[0, 1, 6, 7], [2, 3, 4, 5]]
SCK = [[(1, 0), (9, 1)], [(3, 0), (7, 3)]]   # (nb, local bin offset) per half-set
OUTCOL = [[(0, 256), (768, 1024)], [(256, 512), (512, 768)]]


def _fp8_pair(w):
    hi = np.ascontiguousarray(w).astype(ml_dtypes.float8_e4m3)
    lo = np.ascontiguousarray((w - hi.astype(np.float32)) * 16.0).astype(ml_dtypes.float8_e4m3)
    return hi, lo


def _chunks(total, maxc=512):
    nch = -(-total // maxc)
    base = -(-total // nch)
    out, s = [], 0
    while s < total:
        e = min(s + base, total)
        out.append((s, e - s))
        s = e
    return out


def build_fused(npad, has_bk, has_bv):
    cols = npad * NBK
    cch = _chunks(cols)
    nc = bacc.Bacc("TRN2", target_bir_lowering=False, debug=False, num_devices=8)
    xb = nc.dram_tensor("xb", [D, T], BF16, kind="ExternalInput").ap()
    x8d = nc.dram_tensor("x8", [D, T], F8, kind="ExternalInput").ap()
    wk8d = nc.dram_tensor("wk8", [D, CH], F8, kind="ExternalInput").ap()
    wv = nc.dram_tensor("wvh", [D, CH], BF16, kind="ExternalInput").ap()
    cwd = nc.dram_tensor("cwd", [128, 64], F16, kind="ExternalInput").ap()
    scl8d = nc.dram_tensor("scl8d", [8, 1], F32, kind="ExternalInput").ap()
    rows3d = nc.dram_tensor("rows3", [1, 3 * cols], F32, kind="ExternalInput").ap()
    wtd = nc.dram_tensor("wt", [4 * NBK * 128, 256], F16, kind="ExternalInput").ap()
    cbh = nc.dram_tensor("cbh", [npad, CH], F32, kind="ExternalInput").ap()
    bkb = nc.dram_tensor("bkb", [128, HC], F32, kind="ExternalInput").ap() if has_bk else None
    bvr = nc.dram_tensor("bvr", [128, CH], F32, kind="ExternalInput").ap() if has_bv else None
    out = nc.dram_tensor("out", [npad, CH], F32, kind="ExternalOutput").ap()

    with tile.TileContext(nc) as tc:
        with (
            tc.tile_pool(name="const", bufs=1) as cp,
            tc.tile_pool(name="big", bufs=1) as bigp,
            tc.tile_pool(name="w", bufs=1) as wp,
            tc.tile_pool(name="kc", bufs=6) as kp,
            tc.tile_pool(name="mtp", bufs=2) as mp,
            tc.tile_pool(name="kv", bufs=2, space="PSUM") as kvp,
            tc.tile_pool(name="dn", bufs=1, space="PSUM") as dnp,
            tc.tile_pool(name="pl", bufs=4, space="PSUM") as plp,
        ):
            # ---- small DMAs on the scalar HWDGE ring (need-ordered)
            cw = cp.tile([128, 64], F16)
            nc.scalar.dma_start(cw[:], cwd[:])
            scl8 = cp.tile([8, 1], F32)
            nc.scalar.dma_start(scl8[:], scl8d[:])
            rows3 = cp.tile([1, 3 * cols], F32)
            nc.scalar.dma_start(rows3[:], rows3d[:])

            # ---- big DMAs on the sync HWDGE ring (issue order = need order)
            x8_sb = bigp.tile([128, KT, T], F8, tag="x8")
            x8_r = x8d.rearrange("(k p) t -> p k t", p=128)
            nc.sync.dma_start(x8_sb[:, :, 0:512], x8_r[:, :, 0:512])
            w_k8 = wp.tile([128, KT, CH], F8, tag="wk8")
            nc.sync.dma_start(w_k8[:], wk8d.rearrange("(k p) c -> p k c", p=128))
            nc.sync.dma_start(x8_sb[:, :, 512:T], x8_r[:, :, 512:T])
            w_v = wp.tile([128, KT, CH], BF16, tag="wv")
            nc.sync.dma_start(w_v[:], wv.rearrange("(k p) c -> p k c", p=128))
            x_sb = bigp.tile([128, KT, T], BF16, tag="x")
            x_r = xb.rearrange("(k p) t -> p k t", p=128)
            for c2 in range(T // 512):
                sl = slice(c2 * 512, (c2 + 1) * 512)
                nc.sync.dma_start(x_sb[:, :, sl], x_r[:, :, sl])
            wt_sb = wp.tile([128, 4 * NBK, 256], F16, tag="wt")
            nc.sync.dma_start(wt_sb[:], wtd.rearrange("(w p) o -> p w o", p=128))
            cb_sb = cp.tile([npad, CH], F32)
            nc.scalar.dma_start(cb_sb[:], cbh[:])
            bk_sb = bv_sb = None
            if has_bk:
                bk_sb = cp.tile([128, HC], F32)
                nc.scalar.dma_start(bk_sb[:], bkb[:])
            if has_bv:
                bv_sb = cp.tile([128, CH], F32)
                nc.scalar.dma_start(bv_sb[:], bvr[:])

            # ---- PE warm-up spin: keep the HAM clock gate open while the
            # input DMAs land (otherwise the K stream starts at 1.2 GHz)
            junk = cp.tile([128, 128], BF16)
            nc.gpsimd.memset(junk[:], 0.5)
            wps = kvp.tile([128, 512], F32, tag="kv")
            NWARM = 48
            for i in range(NWARM):
                nc.tensor.matmul(wps[:, 0:128], junk[:], junk[:], start=(i == 0), stop=(i == NWARM - 1))

            ident = cp.tile([128, 128], F32)
            make_identity(nc, ident[:])

            # ---- broadcast bs/be/inv to 128 partitions on gpsimd (idle at
            # start); nbs_bc = p - bs[col], be_bc = be[col] - p in f32.
            # Inside bin: min(t - bs + 1, be - t) >= 1 where t = mt*128 + p.
            tvi = cp.tile([128, 1], I32)
            nc.gpsimd.iota(tvi[:], [[0, 1]], base=0, channel_multiplier=1)
            tv = cp.tile([128, 2], F32)
            bc3 = bigp.tile([128, 3 * cols], F32, tag="bc3")
            nc.gpsimd.partition_broadcast(bc3[:], rows3[:])
            nbs_bc = bc3[:, 0:cols]
            be_bc = bc3[:, cols : 2 * cols]
            inv_bc = bc3[:, 2 * cols : 3 * cols]
            # NOTE: the vector-engine fixups (p - bs, be - p) are emitted
            # after the K section — the vector queue is in-order and they
            # wait on the rows3 DMA + broadcast, which would head-of-line
            # block k2t during K.

            # cw (host-computed): cw[:, ct*16 : ct*16+8] q-mask (col ct = q),
            # [+8:+16] ones-mask (col 4+ct = 1) => dn8 rows 0..3 = dot,
            # 4..7 = nk2; scl8 [8,1] = (1,1,1,1, nq2 x 4)
            mask_sb = bigp.tile([128, MT, cols], F16, tag="mask")

            def emit_mask(mt):
                # indicator(bs <= t < be) = 1{min(t-bs+1, be-t) >= 1} with
                # t = mt*128 + p; bin-count normalization folded in via inv.
                m = float(mt * 128)
                tmp = mp.tile([128, cols], F32, tag="mtmp")
                nc.vector.scalar_tensor_tensor(tmp[:], nbs_bc[:], 2.0 * m + 1.0, be_bc[:], op0=OP.add, op1=OP.min)
                nc.vector.scalar_tensor_tensor(mask_sb[:, mt, :], tmp[:], m + 0.5, inv_bc[:], op0=OP.is_ge, op1=OP.mult)

            # ---- K projection [dk, tok] + dot/nk2 via masked-lhsT matmuls
            dnk = bigp.tile([8, MT, 128], F32, tag="dnk")
            dnk_c = dnk.rearrange("p m t -> p (m t)")
            dcol_d = cp.tile([128, MT, HC], F32)
            dcol_n = cp.tile([128, MT, HC], F32)
            pcol = cp.tile([128, MT, HC], F32)

            def emit_probs(lo, hi):
                Dh = dcol_d[:, lo:hi, :]
                Nh = dcol_n[:, lo:hi, :]
                Ph = pcol[:, lo:hi, :]
                nc.vector.tensor_scalar_max(Nh, Nh, 1e-16)
                nc.scalar.activation(Nh, Nh, AF.Sqrt)
                nc.vector.reciprocal(Nh, Nh)
                nc.vector.tensor_mul(Dh, Dh, Nh)                   # cos
                nc.vector.tensor_scalar_mul(Ph, Dh, -1.0)
                nc.vector.tensor_max(Ph, Ph, Dh)                   # |cos|
                nc.scalar.activation(Ph, Ph, AF.Exp)

            for c2 in range(4):
                dn8 = dnp.tile([8, 512], F32, tag="dn")

                def emit_dn8(ct, ktile, k2t):
                    nc.tensor.matmul(dn8[:], cw[:, ct * 16 : ct * 16 + 8], ktile[:], start=(ct == 0), stop=False)
                    nc.tensor.matmul(dn8[:], cw[:, ct * 16 + 8 : ct * 16 + 16], k2t[:], start=False, stop=(ct == HC - 1))

                # dn8 pairs lag one ct so their lhsT loads can prefetch
                # behind the K-projection matmul stream
                lag = None
                for ct in range(HC):
                    ps = kvp.tile([128, 512], F32, tag="kv")
                    for k in range(0, KT, 2):
                        nc.tensor.matmul(
                            ps[:], w_k8[:, k : k + 2, ct * 128 : (ct + 1) * 128],
                            x8_sb[:, k : k + 2, c2 * 512 : (c2 + 1) * 512],
                            start=(k == 0), stop=(k == KT - 2),
                            perf_mode=mybir.MatmulPerfMode.DoubleRow,
                        )
                    ktile = kp.tile([128, 512], F16, tag="kc")
                    k2t = kp.tile([128, 512], F16, tag="kc")
                    if has_bk:
                        nc.scalar.activation(ktile[:], ps[:], AF.Identity, bias=bk_sb[:, ct : ct + 1])
                        nc.scalar.activation(k2t[:], ps[:], AF.Square, bias=bk_sb[:, ct : ct + 1])
                    else:
                        nc.scalar.activation(ktile[:], ps[:], AF.Copy)
                        nc.vector.tensor_mul(k2t[:], ktile[:], ktile[:])
                    if lag is not None:
                        emit_dn8(*lag)
                    lag = (ct, ktile, k2t)
                emit_dn8(*lag)
                # even rows: dot (x1), odd rows: nk2 (x nq2) via scl8 fold
                nc.scalar.activation(
                    dnk_c[:, c2 * 512 : (c2 + 1) * 512], dn8[:], AF.Copy, scale=scl8[:, 0:1]
                )
                for mt in range(4 * c2, 4 * c2 + 4):
                    tp = dnp.tile([128, 8], F32, tag="tr")
                    nc.tensor.transpose(tp[:], dnk[:, mt, :], ident[0:8, 0:8])
                    nc.vector.tensor_copy(dcol_d[:, mt, :], tp[:, 0:4])
                    nc.vector.tensor_copy(dcol_n[:, mt, :], tp[:, 4:8])
                if c2 % 2 == 1:
                    emit_probs(4 * (c2 - 1), 4 * (c2 + 1))
            # ---- V pass + val = p*v + pooling accumulate (mt-major so the
            #      pooling matmuls overlap the V projection; requires one
            #      PSUM bank per ct held across the whole pass)
            assert len(cch) == 1, "cols > 512 unsupported in fused pooling"
            val_sb = bigp.tile([128, MT, CH], F16, tag="val")
            p_sb = bigp.tile([128, HC, cols], F16, tag="psb")
            p_r = p_sb.rearrange("p c (n i) -> p c n i", i=NBK)
            out_sb = cp.tile([npad, CH], F32)
            pk4 = [plp.tile([128, cols], F32, tag="pl", name=f"pk{i}") for i in range(HC)]
            def emit_pool(mt):
                for ct in range(HC):
                    nc.tensor.matmul(
                        pk4[ct][:], val_sb[:, mt, ct * 128 : (ct + 1) * 128], mask_sb[:, mt, :],
                        start=(mt == 0), stop=(mt == MT - 1),
                    )

            # bs/be broadcast fixups + mask(0) on gpsimd: its queue has no
            # K-section work, so the late rows3 arrival can't head-of-line
            # block the vector queue (the scheduler reorders per engine)
            nc.vector.tensor_copy(tv[:, 0:1], tvi[:])
            nc.vector.tensor_scalar(nbs_bc, nbs_bc, tv[:, 0:1], -1.0, op0=OP.subtract, op1=OP.mult)
            nc.vector.tensor_scalar(be_bc, be_bc, tv[:, 0:1], None, op0=OP.subtract)
            emit_mask(0)

            # pooling matmuls lag one mt behind the V projection so their
            # lhsT (val) is long since written and loads prefetch freely;
            # mask(mt) is emitted during iteration mt, consumed at mt+1
            for mt in range(MT):
                if 1 <= mt <= 15:
                    emit_mask(mt)
                psv = kvp.tile([128, 512], F32, tag="kv")
                for k in range(KT):
                    nc.tensor.matmul(
                        psv[:], x_sb[:, k, mt * 128 : (mt + 1) * 128], w_v[:, k, :],
                        start=(k == 0), stop=(k == KT - 1),
                    )
                src = psv
                if has_bv:
                    tmpv = kp.tile([128, 512], F32, tag="kc")
                    nc.vector.tensor_add(tmpv[:], psv[:], bv_sb[:])
                    src = tmpv
                for ct in range(HC):
                    sl = slice(ct * 128, (ct + 1) * 128)
                    if ct % 2 == 0:
                        nc.vector.tensor_scalar(
                            val_sb[:, mt, sl], src[:, sl], pcol[:, mt, ct : ct + 1], None, op0=OP.mult
                        )
                    else:
                        nc.scalar.activation(
                            val_sb[:, mt, sl], src[:, sl], AF.Copy, scale=pcol[:, mt, ct : ct + 1]
                        )
                if mt >= 1:
                    emit_pool(mt - 1)
            emit_pool(MT - 1)

            # ---- normalize pooled bins + conv per scale group
            for ct in range(HC):
                nc.vector.tensor_copy(p_sb[:, ct, :], pk4[ct][:])
                if ct % 2 == 1:
                    jl = ct // 2
                    po = plp.tile([npad, 256], F32, tag="pl")
                    mms = [(ctl, i) for ctl in range(2) for i in range(NBK)]
                    for idx, (ctl, i) in enumerate(mms):
                        nc.tensor.matmul(
                            po[:], p_r[:, 2 * jl + ctl, :, i], wt_sb[:, jl * 2 * NBK + ctl * NBK + i, :],
                            start=(idx == 0), stop=(idx == len(mms) - 1),
                        )
                    nc.vector.tensor_add(
                        out_sb[:, jl * 256 : (jl + 1) * 256], po[:], cb_sb[:, jl * 256 : (jl + 1) * 256]
                    )
                    nc.sync.dma_start(out[:, jl * 256 : (jl + 1) * 256], out_sb[:, jl * 256 : (jl + 1) * 256])

    nc.compile()
    return nc


def kernel(**inputs):
    global LAST_EXEC_NS, LAST_RES
    LAST_EXEC_NS = 0
    LAST_RES = []
    iv = np.asarray(inputs["input_vectors"], np.float32)
    cls = np.asarray(inputs["clstoken_scales"], np.float32)
    rois = np.asarray(inputs["rois"], np.int32)
    wqT = np.asarray(inputs["Wq"], np.float32).T
    wkT = np.asarray(inputs["Wk"], np.float32).T
    wvT = np.asarray(inputs["Wv"], np.float32).T
    bq = np.asarray(inputs["bq"], np.float32)
    bk = np.asarray(inputs["bk"], np.float32)
    bv = np.asarray(inputs["bv"], np.float32)
    has_bq = bool(np.any(bq))
    has_bk = bool(np.any(bk))
    has_bv = bool(np.any(bv))

    # rois per batch, sorted by start; padded by repeating the last roi
    ords, counts = [], []
    for b in range(B):
        sel = np.nonzero(rois[:, 0] == b)[0]
        if len(sel):
            sel = sel[np.argsort(rois[sel, 1], kind="stable")]
        ords.append(sel)
        counts.append(len(sel))
    npad = max(max(counts), 1)
    cols = npad * NBK

    # per-core channel selections and weight slices; q projection on host
    # (tiny: [B,D]x[D,CH]) -> cw lhsT blocks + scl8 per (batch, half)
    colsel = [np.concatenate([np.arange(h * 128, (h + 1) * 128) for h in hs]) for hs in HSETS]
    q_full = cls @ wqT + bq                                         # [B, D]
    w_slices, cw_cores, scl_cores = [], [], []
    for hh in range(2):
        cs = colsel[hh]
        w_slices.append(
            (
                np.ascontiguousarray(wkT[:, cs] * 16.0).astype(ml_dtypes.float8_e4m3),
                np.ascontiguousarray(wvT[:, cs]).astype(ml_dtypes.bfloat16),
                np.ascontiguousarray(bk[cs] * 16.0),
                np.ascontiguousarray(bv[cs]),
            )
        )
        cwb, sclb = [], []
        for b in range(B):
            qh = q_full[b, cs]                                      # [512]
            cwv = np.zeros((128, 64), np.float16)
            scl = np.ones((8, 1), np.float32)
            for ct in range(HC):
                qc = qh[ct * 128 : (ct + 1) * 128]
                cwv[:, ct * 16 + ct] = qc.astype(np.float16)
                cwv[:, ct * 16 + 8 + 4 + ct] = 1.0
                scl[4 + ct, 0] = float(qc @ qc)
            cwb.append(cwv)
            sclb.append(scl)
        cw_cores.append(cwb)
        scl_cores.append(sclb)

    # conv weights: [jl(2), ctl(2), bin(10)] x [p=128 in-ch, 256 out] zero-padded
    wt_cores, cb_cores = [], []
    for hh in range(2):
        wt = np.zeros((2, 2, NBK, 128, 256), np.float32)
        cbs = []
        for jl, (nb, off) in enumerate(SCK[hh]):
            cwj = np.asarray(inputs[f"conv_w{nb}"], np.float32)      # [o 256, c 256, i nb]
            a = cwj.transpose(1, 2, 0)                               # [c, i, o]
            for ctl in range(2):
                for i in range(nb):
                    wt[jl, ctl, off + i] = a[ctl * 128 : (ctl + 1) * 128, i, :]
            cbs.append(np.asarray(inputs[f"conv_b{nb}"], np.float32))
        wt_cores.append(np.ascontiguousarray(wt.reshape(4 * NBK * 128, 256)).astype(np.float16))
        cb_cores.append(np.ascontiguousarray(np.broadcast_to(np.concatenate(cbs), (npad, CH))))

    ncf = build_fused(npad, has_bk, has_bv)
    in_maps = []
    for core in range(8):
        b, hh = core // 2, core % 2
        sel = ords[b]
        rs = rois[sel] if len(sel) else np.array([[b, 0, 16]], np.int32)
        rs_p = np.concatenate([rs, np.repeat(rs[-1:], npad - len(rs), axis=0)])
        s = rs_p[:, 1].astype(np.int64)
        e = rs_p[:, 2].astype(np.int64)
        L = e - s
        bsa = np.zeros((npad, NBK), np.int64)
        bea = np.zeros((npad, NBK), np.int64)
        for nb, off in SCK[hh]:
            i = np.arange(nb)
            bsa[:, off : off + nb] = s[:, None] + (i[None, :] * L[:, None]) // nb
            bea[:, off : off + nb] = s[:, None] - (-(i[None, :] + 1) * L[:, None]) // nb
        cnt = np.maximum(bea - bsa, 1).astype(np.float32)
        wkh, wvh, bkh, bvh = w_slices[hh]
        m = {
            "xb": np.ascontiguousarray(iv[b]).astype(ml_dtypes.bfloat16),
            "x8": np.ascontiguousarray(iv[b]).astype(ml_dtypes.float8_e4m3),
            "wk8": wkh, "wvh": wvh,
            "cwd": cw_cores[hh][b],
            "scl8d": scl_cores[hh][b],
            "rows3": np.ascontiguousarray(np.concatenate([
                bsa.reshape(cols).astype(np.float32),
                bea.reshape(cols).astype(np.float32),
                (1.0 / cnt).reshape(cols),
            ])[None, :]),
            "wt": wt_cores[hh],
            "cbh": cb_cores[hh],
        }
        if has_bk:
            m["bkb"] = np.ascontiguousarray(bkh.reshape(HC, 128).T)
        if has_bv:
            m["bvr"] = np.ascontiguousarray(np.broadcast_to(bvh, (128, CH)))
        in_maps.append(m)

    r = bass_utils.run_bass_kernel_spmd(ncf, in_maps, core_ids=list(range(8)), trace=TRACE)
    if r.exec_time_ns:
        LAST_EXEC_NS += r.exec_time_ns
    LAST_RES.append(r)

    final = np.empty((NROI, D), np.float32)
    for core in range(8):
        b, hh = core // 2, core % 2
        sel = ords[b]
        if not len(sel):
            continue
        o = r.results[core]["out"]  # [npad, 512]
        for jl in range(2):
            lo, hi = OUTCOL[hh][jl]
            final[sel, lo:hi] = o[: len(sel), jl * 256 : (jl + 1) * 256]
    return final

